# revision 29
# baseline (speedup 1.0000x reference)
"""Grouped-query attention (B=8,S=512,D=4096,G=32) on 8 trn2 cores.

Strategy: data-parallel over the batch dim — core b handles batch b.
Per core, everything is computed in a feature-major ("transposed")
layout so no on-device transposes are needed:

  q^T[f,t] = sum_d WqT[d,f] * xqT[d,t]        (lhsT=WqT tile, rhs=xqT)
  k^T      likewise;  v[t,f] uses lhsT=xqT tile, rhs=WvT tile
  RoPE on q^T/k^T heads 0..7 (per-token angle, head g pairs with g+4)
  s^T[k,q] = kh^T_blk.T @ qh^T   (per head, 4 k-blocks of 128)
  w^T      = exp(s^T + maskbias) (no max-subtraction; logits are O(10))
  o^T[dh,q]= sum_kb vh_blk.T @ w^T_blk        (lhsT=vh block)
  sum[1,q] = ones.T @ w^T  -> r = 1/sum -> broadcast via rank-1 matmul
  attn^T   = o^T * r_bcast  (bf16)
  y[t,f]   = sum_D attnT_blk.T @ WoT tile

Matmuls run in bf16 (fp32 PSUM accumulation); softmax math in fp32.
Host side only shards, transposes (layout), casts dtypes and gathers.
"""

import math

import numpy as np
import ml_dtypes

import concourse.bass as bass
import concourse.mybir as mybir
import concourse.tile as tile
from concourse import bacc
from concourse.bass_utils import run_bass_kernel_spmd
from concourse.tile_rust import add_dep_helper
from concourse import bass_isa

B, S, D = 8, 512, 4096
G, DH = 32, 128
RD = 1024
ALPHA = 1.0 / math.sqrt(DH)
PI = math.pi
NCORES = 8
DT = mybir.dt
AF = mybir.ActivationFunctionType
ALU = mybir.AluOpType

# set by test.py to capture a profile
TRACE = False
LAST_RESULT = None


def _range_reduce(nc, ang, mtmp):
    """In-place reduce ang (>=0, < ~7*pi/2) into (-pi, pi] mod 2*pi."""
    for _ in range(3):
        # mtmp = (ang > pi) * 2pi ; ang -= mtmp
        nc.vector.tensor_scalar(mtmp, ang, PI, 2.0 * PI, ALU.is_gt, ALU.mult)
        nc.vector.tensor_sub(ang, ang, mtmp)


def build_program():
    # Bacc (not plain Bass): its compile pipeline lowers multi-sem waits to
    # the single ISA wait slot; plain Bass BIR fails walrus codegen.
    nc = bacc.Bacc(
        "TRN2", target_bir_lowering=False, debug=False, num_devices=NCORES
    )
    bf16 = DT.bfloat16
    f32 = DT.float32

    # Weights arrive SHARDED: core c holds rows [c*512, (c+1)*512) of each
    # transposed weight (1/8 of the bytes), packed host-side as
    # [8 gq][512 rows][512 cols] so a per-gq column slice is contiguous.
    # On-chip chunked AllGathers rebuild the full matrices in internal
    # DRAM — host->device traffic for weights drops 8x vs replication,
    # and chunking lets the first projection start after the first small
    # gather instead of a full-matrix one.
    WSH = D // NCORES  # 512 rows per shard
    # x inputs pre-packed host-side to [128 part, 32 dblk, S]: one fully
    # contiguous 4MB DMA each instead of strided gathers.
    xqT_d = nc.declare_dram_parameter("xqT", [128, 32, S], bf16, isOutput=False)
    xkT_d = nc.declare_dram_parameter("xkT", [128, 32, S], bf16, isOutput=False)
    xvT_d = nc.declare_dram_parameter("xvT", [128, 32, S], bf16, isOutput=False)
    wq_sh = nc.declare_dram_parameter("wq_sh", [8, WSH, 512], bf16, isOutput=False)
    wk_sh = nc.declare_dram_parameter("wk_sh", [8, WSH, 512], bf16, isOutput=False)
    wv_sh = nc.declare_dram_parameter("wv_sh", [8, WSH, 512], bf16, isOutput=False)
    wo_sh = nc.declare_dram_parameter("wo_sh", [8, WSH, 512], bf16, isOutput=False)
    pos_d = nc.declare_dram_parameter("pos", [S], f32, isOutput=False)
    invf_d = nc.declare_dram_parameter("invf", [S], f32, isOutput=False)
    mask_d = nc.declare_dram_parameter("maskin", [S], DT.int32, isOutput=False)
    # y blocked [fc, tb, 128, 512] so each store is one contiguous DMA;
    # host reassembles to [S, D].
    y_d = nc.declare_dram_parameter("y", [8, 4, 128, 512], bf16, isOutput=True)

    RG = [list(range(NCORES))]

    with tile.TileContext(nc) as tc:
        with tc.tile_pool(name="dram", bufs=1, space="DRAM") as dram:
            # chunk sizes (in gq blocks of 512 cols) per weight: finest for
            # Wq (gates the first matmuls), coarser later to save CC floors.
            # AGs are chained (serialized) so the first chunk's data phase
            # is not delayed behind later collectives' handshakes.
            gathered = {}  # (which, gq) -> (tile, local_j)
            prev_cc = [None]

            def gather_weight(which, sh_d, chunks):
                gq0 = 0
                for ci, csz in enumerate(chunks):
                    bounce = dram.tile(
                        [csz, WSH, 512], bf16,
                        tag=f"b{which}{ci}", name=f"bounce_{which}{ci}",
                    )
                    g_t = dram.tile(
                        [NCORES, csz, WSH, 512], bf16, addr_space="Shared",
                        tag=f"g{which}{ci}", name=f"gath_{which}{ci}",
                    )
                    nc.gpsimd.dma_start(
                        out=bounce, in_=sh_d[gq0 : gq0 + csz]
                    )
                    cc = nc.gpsimd.collective_compute(
                        "AllGather",
                        ALU.bypass,
                        replica_groups=RG,
                        ins=[bounce.opt()],
                        outs=[g_t.opt()],
                    )
                    if prev_cc[0] is not None:
                        add_dep_helper(cc.ins, prev_cc[0].ins, reason="serialize AGs")
                    prev_cc[0] = cc
                    for j in range(csz):
                        gathered[(which, gq0 + j)] = (g_t, j)
                    gq0 += csz

            gather_weight("q", wq_sh, (1, 1, 1, 1, 2, 2))
            gather_weight("k", wk_sh, (2, 2, 2, 2))
            gather_weight("v", wv_sh, (4, 4))
            gather_weight("o", wo_sh, (8,))

            def w_tile(which, gq, db):
                """[128, 512] tile of wT[db*128:(db+1)*128, gq cols]."""
                g_t, j = gathered[(which, gq)]
                r, lb = db // 4, db % 4
                return g_t[r, j, lb * 128 : (lb + 1) * 128, :]

            def w_src(which, gq, h, lb):
                """Strided 3-dim AP [128, 8, 512]: d-blocks {r*4 + 2h + lb}
                for r in 0..7 of column block gq (DMA APs max 3 dims)."""
                g_t, j = gathered[(which, gq)]
                r0 = h * 256 + lb * 128
                src = g_t[:, j, r0 : r0 + 128, :]
                return src.rearrange("r p f -> p r f", p=128)

            _build_body(nc, tc, xqT_d, xkT_d, xvT_d, w_src, w_tile,
                        pos_d, invf_d, mask_d, y_d)
    # Bacc lowering: splits multi-sem waits into the single ISA wait slot,
    # allocates registers, fuses nops. Required before walrus codegen.
    nc.compile()
    return nc


def _build_body(nc, tc, xqT_d, xkT_d, xvT_d, w_src, w_tile,
                pos_d, invf_d, mask_d, y_d):
    bf16 = DT.bfloat16
    f32 = DT.float32
    if True:
        with tc.tile_pool(name="persist", bufs=1) as persist:
            # trig tiles, broadcast over partitions: [128, S]
            sin_t = persist.tile([128, S], f32, tag="sin")
            cos_t = persist.tile([128, S], f32, tag="cos")
            sinq_t = persist.tile([128, S], f32, tag="sinq")
            cosq_t = persist.tile([128, S], f32, tag="cosq")
            maskb = persist.tile([128, 4], f32, tag="maskb")
            ones_bf = persist.tile([128, 1], bf16, tag="ones_bf")
            ones_f1 = persist.tile([1, 128], f32, tag="ones_f1")
            qT_s = persist.tile([128, G, S], bf16, tag="qT")
            kT_s = persist.tile([128, G, S], bf16, tag="kT")
            v_s = persist.tile([128, 4, G, DH], bf16, tag="v")

            nc.vector.memset(ones_bf, 1.0)
            nc.vector.memset(ones_f1, 1.0)

            # ---- setup: trig + mask ----
            # Trig is computed on one partition, then broadcast to all 128
            # partitions with a rank-1 matmul (ones [1,128] x row [1,S]).
            with (
                tc.tile_pool(name="setup", bufs=1) as setup,
                tc.tile_pool(name="ps_setup", bufs=2, space="PSUM") as ps_setup,
            ):
                pos1 = setup.tile([1, S], f32, tag="pos1")
                invf1 = setup.tile([1, S], f32, tag="invf1")
                angc = setup.tile([1, S], f32, tag="angc")
                mtmp = setup.tile([1, S], f32, tag="mtmp")
                mi = setup.tile([128, 4], DT.int32, tag="mi")
                mf = setup.tile([128, 4], f32, tag="mf")

                nc.sync.dma_start(out=pos1, in_=pos_d[None, :])
                nc.sync.dma_start(out=invf1, in_=invf_d[None, :])
                # angles for sin, reuse pos1 as buffer for sin-angles
                angs = pos1
                nc.vector.tensor_mul(angs, pos1, invf1)
                nc.vector.tensor_scalar_add(angc, angs, PI / 2.0)
                _range_reduce(nc, angs, mtmp)
                _range_reduce(nc, angc, mtmp)
                nc.scalar.activation(angs, angs, AF.Sin)
                nc.scalar.activation(angc, angc, AF.Sin)
                ps_sin = ps_setup.tile([128, S], f32, tag="b", name="ps_sin")
                ps_cos = ps_setup.tile([128, S], f32, tag="b", name="ps_cos")
                nc.tensor.matmul(ps_sin, ones_f1, angs, start=True, stop=True)
                nc.tensor.matmul(ps_cos, ones_f1, angc, start=True, stop=True)
                nc.scalar.copy(sin_t, ps_sin)
                nc.scalar.copy(cos_t, ps_cos)
                nc.scalar.mul(sinq_t, ps_sin, ALPHA)
                nc.scalar.mul(cosq_t, ps_cos, ALPHA)

                nc.sync.dma_start(out=mi, in_=mask_d[:].rearrange("(b p) -> p b", p=128))
                nc.vector.tensor_copy(mf, mi)
                # maskb = m * 1e9 - 1e9  (0 where m==1, -1e9 where m==0)
                nc.vector.tensor_scalar(
                    maskb, mf, 1.0e9, 1.0e9, ALU.mult, ALU.subtract
                )

            # ---- q and k projections (output feature-major) + RoPE ----
            for which, xT_d, outT, ct, st in (
                ("q", xqT_d, qT_s, cosq_t, sinq_t),
                ("k", xkT_d, kT_s, cos_t, sin_t),
            ):
                scale = ALPHA if which == "q" else 1.0
                with (
                    tc.tile_pool(name=f"x{which}", bufs=1) as xin_pool,
                    tc.tile_pool(name=f"w{which}", bufs=2) as w_pool,
                    tc.tile_pool(name=f"ps{which}", bufs=8, space="PSUM") as ps_pool,
                    tc.tile_pool(name=f"rope{which}", bufs=1) as rope_pool,
                    tc.tile_pool(name=f"rt{which}", bufs=4) as rtmp_pool,
                ):
                    xT_s = xin_pool.tile([128, 32, S], bf16, tag="xT")
                    nc.sync.dma_start(out=xT_s, in_=xT_d[:])
                    rope_f32 = rope_pool.tile([128, 8, S], f32, tag="rope")
                    for gq in range(8):
                        pss = [
                            ps_pool.tile([128, S], f32, tag="ps", name="ps_qk") for _ in range(4)
                        ]
                        for h in range(2):
                            wst = w_pool.tile([128, 8, 2, 512], bf16, tag="w")
                            for lb2 in range(2):
                                nc.sync.dma_start(
                                    out=wst[:, :, lb2, :],
                                    in_=w_src(which, gq, h, lb2),
                                )
                            for i in range(16):
                                r, lb = i // 2, i % 2
                                db = r * 4 + 2 * h + lb
                                for g4 in range(4):
                                    nc.tensor.matmul(
                                        pss[g4],
                                        wst[:, r, lb, g4 * 128 : (g4 + 1) * 128],
                                        xT_s[:, db, :],
                                        start=(h == 0 and i == 0),
                                        stop=(h == 1 and i == 15),
                                    )
                        for g4 in range(4):
                            g = gq * 4 + g4
                            if g < 8:
                                # RoPE heads: stash fp32
                                nc.vector.tensor_copy(rope_f32[:, g, :], pss[g4])
                            elif which == "q":
                                nc.scalar.activation(
                                    outT[:, g, :], pss[g4], AF.Copy, scale=scale
                                )
                            else:
                                nc.vector.tensor_copy(outT[:, g, :], pss[g4])
                    # RoPE: head g pairs with head g+4 (per-token scalar angle)
                    for g in range(4):
                        a = rope_f32[:, g, :]
                        b = rope_f32[:, g + 4, :]
                        t1 = rtmp_pool.tile([128, S], f32, tag="t")
                        t2 = rtmp_pool.tile([128, S], f32, tag="t")
                        t3 = rtmp_pool.tile([128, S], f32, tag="t")
                        t4 = rtmp_pool.tile([128, S], f32, tag="t")
                        nc.vector.tensor_mul(t1, a, ct)
                        nc.vector.tensor_mul(t2, b, st)
                        nc.vector.tensor_sub(outT[:, g, :], t1, t2)
                        nc.vector.tensor_mul(t3, b, ct)
                        nc.vector.tensor_mul(t4, a, st)
                        nc.vector.tensor_add(outT[:, g + 4, :], t3, t4)

            # ---- v projection + attention, interleaved per fc block ----
            # Attention needs no gathered weights, so it is PE filler work
            # the scheduler can run whenever v's AllGathered chunks lag.
            # Softmax denominators go through GpSimd partition_all_reduce
            # (idle engine) instead of ones-matmuls: frees 4 PSUM banks
            # and 5 small matmuls per head.
            with tc.tile_pool(name="attn", bufs=1) as attn_pool:
                attnT_s = attn_pool.tile([128, G, S], bf16, tag="attnT")
                with (
                    tc.tile_pool(name="xv", bufs=1) as xin_pool,
                    tc.tile_pool(name="wv", bufs=12) as w_pool,
                    tc.tile_pool(name="psv", bufs=4, space="PSUM") as ps_pool,
                    tc.tile_pool(name="wexp", bufs=6) as wexp_pool,
                    tc.tile_pool(name="rsm", bufs=6) as rpool,
                    tc.tile_pool(name="ps_s", bufs=2, space="PSUM") as ps_s_pool,
                    tc.tile_pool(name="ps_o", bufs=2, space="PSUM") as ps_o_pool,
                ):
                    xT_s = xin_pool.tile([128, 32, S], bf16, tag="xT")
                    nc.sync.dma_start(out=xT_s, in_=xvT_d[:])
                    for fc in range(8):
                        pss = [ps_pool.tile([128, 512], f32, tag="ps", name="ps_v") for _ in range(4)]
                        for d in range(32):
                            wt = w_pool.tile([128, 512], bf16, tag="w")
                            nc.sync.dma_start(out=wt, in_=w_tile("v", fc, d))
                            for tb in range(4):
                                nc.tensor.matmul(
                                    pss[tb],
                                    xT_s[:, d, tb * 128 : (tb + 1) * 128],
                                    wt,
                                    start=(d == 0),
                                    stop=(d == 31),
                                )
                        for tb in range(4):
                            nc.vector.tensor_copy(
                                v_s[:, tb, fc * 4 : (fc + 1) * 4, :], pss[tb]
                            )
                        for g in range(fc * 4, fc * 4 + 4):
                            ps_o = ps_o_pool.tile([128, S], f32, tag="o")
                            wbs = []
                            for kb in range(4):
                                ps_sc = ps_s_pool.tile([128, S], f32, tag="s")
                                nc.tensor.matmul(
                                    ps_sc,
                                    kT_s[:, g, kb * 128 : (kb + 1) * 128],
                                    qT_s[:, g, :],
                                    start=True,
                                    stop=True,
                                )
                                wb = wexp_pool.tile([128, S], bf16, tag="w")
                                nc.scalar.activation(
                                    wb, ps_sc, AF.Exp,
                                    bias=maskb[:, kb : kb + 1], scale=1.0,
                                )
                                nc.tensor.matmul(
                                    ps_o,
                                    v_s[:, kb, g, :],
                                    wb,
                                    start=(kb == 0),
                                    stop=(kb == 3),
                                )
                                wbs.append(wb)
                            s01 = rpool.tile([128, S], f32, tag="t")
                            s23 = rpool.tile([128, S], f32, tag="t")
                            stot = rpool.tile([128, S], f32, tag="t")
                            nc.vector.tensor_add(s01, wbs[0], wbs[1])
                            nc.vector.tensor_add(s23, wbs[2], wbs[3])
                            nc.vector.tensor_add(stot, s01, s23)
                            rsum = rpool.tile([128, S], f32, tag="t")
                            nc.gpsimd.partition_all_reduce(
                                rsum, stot, 128, bass_isa.ReduceOp.add
                            )
                            # reciprocal on ONE partition row (iterative op,
                            # 8x per-element cost), then GpSimd-broadcast
                            rrow = rpool.tile([1, S], f32, tag="rr", bufs=2)
                            nc.vector.reciprocal(rrow, rsum[0:1, :])
                            rinv = rpool.tile([128, S], f32, tag="ri", bufs=2)
                            nc.gpsimd.partition_broadcast(rinv, rrow)
                            nc.vector.tensor_mul(attnT_s[:, g, :], ps_o, rinv)

                # ---- y = attn @ Wo.T  (token-major output) ----
                with (
                    tc.tile_pool(name="wo", bufs=3) as wo_pool,
                    tc.tile_pool(name="psy", bufs=8, space="PSUM") as psy_pool,
                    tc.tile_pool(name="yout", bufs=4) as y_pool,
                ):
                    for fc in range(8):
                        pss = [
                            psy_pool.tile([128, 512], f32, tag="ps", name="ps_y") for _ in range(4)
                        ]
                        for h in range(2):
                            wst = wo_pool.tile([128, 8, 2, 512], bf16, tag="w")
                            for lb2 in range(2):
                                nc.sync.dma_start(
                                    out=wst[:, :, lb2, :], in_=w_src("o", fc, h, lb2)
                                )
                            for i in range(16):
                                r, lb = i // 2, i % 2
                                dD = r * 4 + 2 * h + lb
                                for tb in range(4):
                                    nc.tensor.matmul(
                                        pss[tb],
                                        attnT_s[:, dD, tb * 128 : (tb + 1) * 128],
                                        wst[:, r, lb, :],
                                        start=(h == 0 and i == 0),
                                        stop=(h == 1 and i == 15),
                                    )
                        for tb in range(4):
                            yt = y_pool.tile([128, 512], bf16, tag="y")
                            nc.vector.tensor_copy(yt, pss[tb])
                            nc.sync.dma_start(
                                out=y_d[fc, tb], in_=yt
                            )


_NC_CACHE = None


def _get_program():
    global _NC_CACHE
    if _NC_CACHE is None:
        _NC_CACHE = build_program()
    return _NC_CACHE


def make_in_maps(query, key, value, mask, position_ids, Wq, Wk, Wv, Wo):
    bf16 = ml_dtypes.bfloat16
    WSH = D // NCORES

    def t_bf16(a):  # [m,n] fp32 -> [n,m] bf16, contiguous
        return np.asarray(a, np.float32).T.astype(bf16)

    wqT = t_bf16(np.asarray(Wq))
    wkT = t_bf16(np.asarray(Wk))
    wvT = t_bf16(np.asarray(Wv))
    woT = t_bf16(np.asarray(Wo))
    invf = (10000.0 ** (-np.arange(0, RD, 2, dtype=np.float32) / RD)).astype(
        np.float32
    )

    def shard(wT, b):
        # rows [b*512,(b+1)*512), packed [8 gq][512 rows][512 cols] so each
        # per-gq column slice is one contiguous AllGather input chunk
        s = wT[b * WSH : (b + 1) * WSH]  # [512, 4096]
        return np.ascontiguousarray(s.reshape(WSH, 8, 512).transpose(1, 0, 2))

    def pack_x(a):
        # [512 tok, 4096 d] fp32 -> [128 part, 32 dblk, 512 tok] bf16
        xT = t_bf16(a)  # [4096, 512]
        return np.ascontiguousarray(
            xT.reshape(32, 128, 512).transpose(1, 0, 2)
        )

    in_maps = []
    for b in range(NCORES):
        in_maps.append(
            {
                "xqT": pack_x(query[b]),
                "xkT": pack_x(key[b]),
                "xvT": pack_x(value[b]),
                # core b ships only its row-shard; the kernel AllGathers
                "wq_sh": shard(wqT, b),
                "wk_sh": shard(wkT, b),
                "wv_sh": shard(wvT, b),
                "wo_sh": shard(woT, b),
                "pos": np.ascontiguousarray(
                    np.asarray(position_ids[b], np.float32)
                ),
                "invf": invf,
                "maskin": np.ascontiguousarray(np.asarray(mask[b], np.int32)),
            }
        )
    return in_maps


def kernel(query, key, value, mask, position_ids, Wq, Wk, Wv, Wo):
    global LAST_RESULT
    nc = _get_program()
    in_maps = make_in_maps(
        query, key, value, mask, position_ids, Wq, Wk, Wv, Wo
    )
    res = run_bass_kernel_spmd(
        nc, in_maps, core_ids=list(range(NCORES)), trace=TRACE
    )
    LAST_RESULT = res
    # y comes back bf16 and fc/tb-blocked; reassemble + widen host-side.
    outs = []
    for b in range(NCORES):
        blk = np.asarray(res.results[b]["y"])  # [8 fc, 4 tb, 128, 512]
        outs.append(blk.transpose(1, 2, 0, 3).reshape(S, D))
    out = np.stack(outs, axis=0)
    return np.ascontiguousarray(out.astype(np.float32))



# revision 31
# speedup vs baseline: 1.0596x; 1.0596x over previous
"""Grouped-query attention (B=8,S=512,D=4096,G=32) on 8 trn2 cores.

Strategy: data-parallel over the batch dim — core b handles batch b —
with the WEIGHTS SHARDED across cores and AllGathered on-chip. Each
core ships only rows [c*512,(c+1)*512) of each transposed weight
(16.8MB instead of 134MB per core), cutting total host->device input
bytes ~5x; chunked, serialized AllGathers rebuild the matrices in
internal Shared DRAM while compute runs, ordered to match consumption
(Wq first, per-gq chunks, Wo last).

Per core, everything is computed in a feature-major ("transposed")
layout so no on-device transposes are needed:

  q^T[f,t] = sum_d WqT[d,f] * xqT[d,t]        (lhsT=WqT tile, rhs=xqT)
  k^T      likewise;  v[t,f] uses lhsT=xqT tile, rhs=WvT tile
  RoPE on q^T/k^T heads 0..7 (per-token angle, head g pairs with g+4)
  s^T[k,q] = kh^T_blk.T @ qh^T   (per head, 4 k-blocks of 128)
  w^T      = exp(s^T + maskbias) (no max-subtraction; logits are O(10))
  o^T[dh,q]= sum_kb vh_blk.T @ w^T_blk        (lhsT=vh block)
  denom    = DVE-sum of w^T blocks -> GpSimd partition_all_reduce
  r        = 1/denom on one row -> GpSimd partition_broadcast
  attn^T   = o^T * r  (bf16)
  y[t,f]   = sum_D attnT_blk.T @ WoT tile

The v-projection and attention are interleaved per 4-head block:
attention needs no gathered weights, so it is filler work the
scheduler can run whenever a weight AllGather chunk lags the PE.
Matmuls run in bf16 (fp32 PSUM accumulation); softmax math in fp32;
y returns bf16 (host widens) to halve the d2h bytes.
Host side only shards, transposes (layout), casts dtypes and gathers.
"""

import math

import numpy as np
import ml_dtypes

import concourse.bass as bass
import concourse.mybir as mybir
import concourse.tile as tile
from concourse import bacc
from concourse.bass_utils import run_bass_kernel_spmd
from concourse.tile_rust import add_dep_helper
from concourse import bass_isa

B, S, D = 8, 512, 4096
G, DH = 32, 128
RD = 1024
ALPHA = 1.0 / math.sqrt(DH)
PI = math.pi
NCORES = 8
DT = mybir.dt
AF = mybir.ActivationFunctionType
ALU = mybir.AluOpType

# set by test.py to capture a profile
TRACE = False
LAST_RESULT = None


def _range_reduce(nc, ang, mtmp):
    """In-place reduce ang (>=0, < ~7*pi/2) into (-pi, pi] mod 2*pi."""
    for _ in range(3):
        # mtmp = (ang > pi) * 2pi ; ang -= mtmp
        nc.vector.tensor_scalar(mtmp, ang, PI, 2.0 * PI, ALU.is_gt, ALU.mult)
        nc.vector.tensor_sub(ang, ang, mtmp)


def build_program():
    # Bacc (not plain Bass): its compile pipeline lowers multi-sem waits to
    # the single ISA wait slot; plain Bass BIR fails walrus codegen.
    nc = bacc.Bacc(
        "TRN2", target_bir_lowering=False, debug=False, num_devices=NCORES
    )
    bf16 = DT.bfloat16
    f32 = DT.float32

    # Weights arrive SHARDED: core c holds rows [c*512, (c+1)*512) of each
    # transposed weight (1/8 of the bytes), packed host-side as
    # [8 gq][512 rows][512 cols] so a per-gq column slice is contiguous.
    # On-chip chunked AllGathers rebuild the full matrices in internal
    # DRAM — host->device traffic for weights drops 8x vs replication,
    # and chunking lets the first projection start after the first small
    # gather instead of a full-matrix one.
    WSH = D // NCORES  # 512 rows per shard
    # x inputs pre-packed host-side to [128 part, 32 dblk, S]: one fully
    # contiguous 4MB DMA each instead of strided gathers.
    xqT_d = nc.declare_dram_parameter("xqT", [128, 32, S], bf16, isOutput=False)
    xkT_d = nc.declare_dram_parameter("xkT", [128, 32, S], bf16, isOutput=False)
    xvT_d = nc.declare_dram_parameter("xvT", [128, 32, S], bf16, isOutput=False)
    wq_sh = nc.declare_dram_parameter("wq_sh", [8, WSH, 512], bf16, isOutput=False)
    wk_sh = nc.declare_dram_parameter("wk_sh", [8, WSH, 512], bf16, isOutput=False)
    wv_sh = nc.declare_dram_parameter("wv_sh", [8, WSH, 512], bf16, isOutput=False)
    wo_sh = nc.declare_dram_parameter("wo_sh", [8, WSH, 512], bf16, isOutput=False)
    pos_d = nc.declare_dram_parameter("pos", [S], f32, isOutput=False)
    invf_d = nc.declare_dram_parameter("invf", [S], f32, isOutput=False)
    mask_d = nc.declare_dram_parameter("maskin", [S], DT.int32, isOutput=False)
    # y blocked [fc, tb, 128, 512] so each store is one contiguous DMA;
    # host reassembles to [S, D].
    y_d = nc.declare_dram_parameter("y", [8, 4, 128, 512], bf16, isOutput=True)

    RG = [list(range(NCORES))]

    with tile.TileContext(nc) as tc:
        with tc.tile_pool(name="dram", bufs=1, space="DRAM") as dram:
            # chunk sizes (in gq blocks of 512 cols) per weight: finest for
            # Wq (gates the first matmuls), coarser later to save CC floors.
            # AGs are chained (serialized) so the first chunk's data phase
            # is not delayed behind later collectives' handshakes.
            gathered = {}  # (which, gq) -> (tile, local_j)
            prev_cc = [None]

            def gather_weight(which, sh_d, chunks):
                gq0 = 0
                for ci, csz in enumerate(chunks):
                    bounce = dram.tile(
                        [csz, WSH, 512], bf16,
                        tag=f"b{which}{ci}", name=f"bounce_{which}{ci}",
                    )
                    g_t = dram.tile(
                        [NCORES, csz, WSH, 512], bf16, addr_space="Shared",
                        tag=f"g{which}{ci}", name=f"gath_{which}{ci}",
                    )
                    nc.gpsimd.dma_start(
                        out=bounce, in_=sh_d[gq0 : gq0 + csz]
                    )
                    cc = nc.gpsimd.collective_compute(
                        "AllGather",
                        ALU.bypass,
                        replica_groups=RG,
                        ins=[bounce.opt()],
                        outs=[g_t.opt()],
                    )
                    if prev_cc[0] is not None:
                        add_dep_helper(cc.ins, prev_cc[0].ins, reason="serialize AGs")
                    prev_cc[0] = cc
                    for j in range(csz):
                        gathered[(which, gq0 + j)] = (g_t, j)
                    gq0 += csz

            gather_weight("q", wq_sh, (1, 1, 2, 2, 2))
            gather_weight("k", wk_sh, (2, 2, 2, 2))
            gather_weight("v", wv_sh, (8,))
            gather_weight("o", wo_sh, (8,))

            def w_tile(which, gq, db):
                """[128, 512] tile of wT[db*128:(db+1)*128, gq cols]."""
                g_t, j = gathered[(which, gq)]
                r, lb = db // 4, db % 4
                return g_t[r, j, lb * 128 : (lb + 1) * 128, :]

            def w_src(which, gq, h, lb):
                """Strided 3-dim AP [128, 8, 512]: d-blocks {r*4 + 2h + lb}
                for r in 0..7 of column block gq (DMA APs max 3 dims)."""
                g_t, j = gathered[(which, gq)]
                r0 = h * 256 + lb * 128
                src = g_t[:, j, r0 : r0 + 128, :]
                return src.rearrange("r p f -> p r f", p=128)

            _build_body(nc, tc, xqT_d, xkT_d, xvT_d, w_src, w_tile,
                        pos_d, invf_d, mask_d, y_d)
    # Bacc lowering: splits multi-sem waits into the single ISA wait slot,
    # allocates registers, fuses nops. Required before walrus codegen.
    nc.compile()
    return nc


def _build_body(nc, tc, xqT_d, xkT_d, xvT_d, w_src, w_tile,
                pos_d, invf_d, mask_d, y_d):
    bf16 = DT.bfloat16
    f32 = DT.float32
    if True:
        with tc.tile_pool(name="persist", bufs=1) as persist:
            # trig tiles, broadcast over partitions: [128, S]
            sin_t = persist.tile([128, S], f32, tag="sin")
            cos_t = persist.tile([128, S], f32, tag="cos")
            sinq_t = persist.tile([128, S], f32, tag="sinq")
            cosq_t = persist.tile([128, S], f32, tag="cosq")
            maskb = persist.tile([128, 4], f32, tag="maskb")
            ones_bf = persist.tile([128, 1], bf16, tag="ones_bf")
            ones_f1 = persist.tile([1, 128], f32, tag="ones_f1")
            qT_s = persist.tile([128, G, S], bf16, tag="qT")
            kT_s = persist.tile([128, G, S], bf16, tag="kT")
            v_s = persist.tile([128, 4, G, DH], bf16, tag="v")

            nc.vector.memset(ones_bf, 1.0)
            nc.vector.memset(ones_f1, 1.0)

            # ---- setup: trig + mask ----
            # Trig is computed on one partition, then broadcast to all 128
            # partitions with a rank-1 matmul (ones [1,128] x row [1,S]).
            with (
                tc.tile_pool(name="setup", bufs=1) as setup,
                tc.tile_pool(name="ps_setup", bufs=2, space="PSUM") as ps_setup,
            ):
                pos1 = setup.tile([1, S], f32, tag="pos1")
                invf1 = setup.tile([1, S], f32, tag="invf1")
                angc = setup.tile([1, S], f32, tag="angc")
                mtmp = setup.tile([1, S], f32, tag="mtmp")
                mi = setup.tile([128, 4], DT.int32, tag="mi")
                mf = setup.tile([128, 4], f32, tag="mf")

                nc.sync.dma_start(out=pos1, in_=pos_d[None, :])
                nc.sync.dma_start(out=invf1, in_=invf_d[None, :])
                # angles for sin, reuse pos1 as buffer for sin-angles
                angs = pos1
                nc.vector.tensor_mul(angs, pos1, invf1)
                nc.vector.tensor_scalar_add(angc, angs, PI / 2.0)
                _range_reduce(nc, angs, mtmp)
                _range_reduce(nc, angc, mtmp)
                nc.scalar.activation(angs, angs, AF.Sin)
                nc.scalar.activation(angc, angc, AF.Sin)
                ps_sin = ps_setup.tile([128, S], f32, tag="b", name="ps_sin")
                ps_cos = ps_setup.tile([128, S], f32, tag="b", name="ps_cos")
                nc.tensor.matmul(ps_sin, ones_f1, angs, start=True, stop=True)
                nc.tensor.matmul(ps_cos, ones_f1, angc, start=True, stop=True)
                nc.scalar.copy(sin_t, ps_sin)
                nc.scalar.copy(cos_t, ps_cos)
                nc.scalar.mul(sinq_t, ps_sin, ALPHA)
                nc.scalar.mul(cosq_t, ps_cos, ALPHA)

                nc.sync.dma_start(out=mi, in_=mask_d[:].rearrange("(b p) -> p b", p=128))
                nc.vector.tensor_copy(mf, mi)
                # maskb = m * 1e9 - 1e9  (0 where m==1, -1e9 where m==0)
                nc.vector.tensor_scalar(
                    maskb, mf, 1.0e9, 1.0e9, ALU.mult, ALU.subtract
                )

            # ---- q and k projections (output feature-major) + RoPE ----
            for which, xT_d, outT, ct, st in (
                ("q", xqT_d, qT_s, cosq_t, sinq_t),
                ("k", xkT_d, kT_s, cos_t, sin_t),
            ):
                scale = ALPHA if which == "q" else 1.0
                with (
                    tc.tile_pool(name=f"x{which}", bufs=1) as xin_pool,
                    tc.tile_pool(name=f"w{which}", bufs=2) as w_pool,
                    tc.tile_pool(name=f"ps{which}", bufs=8, space="PSUM") as ps_pool,
                    tc.tile_pool(name=f"rope{which}", bufs=1) as rope_pool,
                    tc.tile_pool(name=f"rt{which}", bufs=4) as rtmp_pool,
                ):
                    xT_s = xin_pool.tile([128, 32, S], bf16, tag="xT")
                    nc.sync.dma_start(out=xT_s, in_=xT_d[:])
                    rope_f32 = rope_pool.tile([128, 8, S], f32, tag="rope")
                    for gq in range(8):
                        pss = [
                            ps_pool.tile([128, S], f32, tag="ps", name="ps_qk") for _ in range(4)
                        ]
                        for h in range(2):
                            wst = w_pool.tile([128, 8, 2, 512], bf16, tag="w")
                            for lb2 in range(2):
                                nc.sync.dma_start(
                                    out=wst[:, :, lb2, :],
                                    in_=w_src(which, gq, h, lb2),
                                )
                            for i in range(16):
                                r, lb = i // 2, i % 2
                                db = r * 4 + 2 * h + lb
                                for g4 in range(4):
                                    nc.tensor.matmul(
                                        pss[g4],
                                        wst[:, r, lb, g4 * 128 : (g4 + 1) * 128],
                                        xT_s[:, db, :],
                                        start=(h == 0 and i == 0),
                                        stop=(h == 1 and i == 15),
                                    )
                        for g4 in range(4):
                            g = gq * 4 + g4
                            if g < 8:
                                # RoPE heads: stash fp32
                                nc.vector.tensor_copy(rope_f32[:, g, :], pss[g4])
                            elif which == "q":
                                nc.scalar.activation(
                                    outT[:, g, :], pss[g4], AF.Copy, scale=scale
                                )
                            else:
                                nc.vector.tensor_copy(outT[:, g, :], pss[g4])
                    # RoPE: head g pairs with head g+4 (per-token scalar angle)
                    for g in range(4):
                        a = rope_f32[:, g, :]
                        b = rope_f32[:, g + 4, :]
                        t1 = rtmp_pool.tile([128, S], f32, tag="t")
                        t2 = rtmp_pool.tile([128, S], f32, tag="t")
                        t3 = rtmp_pool.tile([128, S], f32, tag="t")
                        t4 = rtmp_pool.tile([128, S], f32, tag="t")
                        nc.vector.tensor_mul(t1, a, ct)
                        nc.vector.tensor_mul(t2, b, st)
                        nc.vector.tensor_sub(outT[:, g, :], t1, t2)
                        nc.vector.tensor_mul(t3, b, ct)
                        nc.vector.tensor_mul(t4, a, st)
                        nc.vector.tensor_add(outT[:, g + 4, :], t3, t4)

            # ---- v projection + attention, interleaved per fc block ----
            # Attention needs no gathered weights, so it is PE filler work
            # the scheduler can run whenever v's AllGathered chunks lag.
            # Softmax denominators go through GpSimd partition_all_reduce
            # (idle engine) instead of ones-matmuls: frees 4 PSUM banks
            # and 5 small matmuls per head.
            with tc.tile_pool(name="attn", bufs=1) as attn_pool:
                attnT_s = attn_pool.tile([128, G, S], bf16, tag="attnT")
                with (
                    tc.tile_pool(name="xv", bufs=1) as xin_pool,
                    tc.tile_pool(name="wv", bufs=12) as w_pool,
                    tc.tile_pool(name="psv", bufs=4, space="PSUM") as ps_pool,
                    tc.tile_pool(name="wexp", bufs=6) as wexp_pool,
                    tc.tile_pool(name="rsm", bufs=6) as rpool,
                    tc.tile_pool(name="ps_s", bufs=2, space="PSUM") as ps_s_pool,
                    tc.tile_pool(name="ps_o", bufs=2, space="PSUM") as ps_o_pool,
                ):
                    xT_s = xin_pool.tile([128, 32, S], bf16, tag="xT")
                    nc.sync.dma_start(out=xT_s, in_=xvT_d[:])
                    for fc in range(8):
                        pss = [ps_pool.tile([128, 512], f32, tag="ps", name="ps_v") for _ in range(4)]
                        for d in range(32):
                            wt = w_pool.tile([128, 512], bf16, tag="w")
                            nc.sync.dma_start(out=wt, in_=w_tile("v", fc, d))
                            for tb in range(4):
                                nc.tensor.matmul(
                                    pss[tb],
                                    xT_s[:, d, tb * 128 : (tb + 1) * 128],
                                    wt,
                                    start=(d == 0),
                                    stop=(d == 31),
                                )
                        for tb in range(4):
                            nc.vector.tensor_copy(
                                v_s[:, tb, fc * 4 : (fc + 1) * 4, :], pss[tb]
                            )
                        for g in range(fc * 4, fc * 4 + 4):
                            ps_o = ps_o_pool.tile([128, S], f32, tag="o")
                            wbs = []
                            for kb in range(4):
                                ps_sc = ps_s_pool.tile([128, S], f32, tag="s")
                                nc.tensor.matmul(
                                    ps_sc,
                                    kT_s[:, g, kb * 128 : (kb + 1) * 128],
                                    qT_s[:, g, :],
                                    start=True,
                                    stop=True,
                                )
                                wb = wexp_pool.tile([128, S], bf16, tag="w")
                                nc.scalar.activation(
                                    wb, ps_sc, AF.Exp,
                                    bias=maskb[:, kb : kb + 1], scale=1.0,
                                )
                                nc.tensor.matmul(
                                    ps_o,
                                    v_s[:, kb, g, :],
                                    wb,
                                    start=(kb == 0),
                                    stop=(kb == 3),
                                )
                                wbs.append(wb)
                            s01 = rpool.tile([128, S], f32, tag="t")
                            s23 = rpool.tile([128, S], f32, tag="t")
                            stot = rpool.tile([128, S], f32, tag="t")
                            nc.vector.tensor_add(s01, wbs[0], wbs[1])
                            nc.vector.tensor_add(s23, wbs[2], wbs[3])
                            nc.vector.tensor_add(stot, s01, s23)
                            rsum = rpool.tile([128, S], f32, tag="t")
                            nc.gpsimd.partition_all_reduce(
                                rsum, stot, 128, bass_isa.ReduceOp.add
                            )
                            # reciprocal on ONE partition row (iterative op,
                            # 8x per-element cost), then GpSimd-broadcast
                            rrow = rpool.tile([1, S], f32, tag="rr", bufs=2)
                            nc.vector.reciprocal(rrow, rsum[0:1, :])
                            rinv = rpool.tile([128, S], f32, tag="ri", bufs=2)
                            nc.gpsimd.partition_broadcast(rinv, rrow)
                            nc.vector.tensor_mul(attnT_s[:, g, :], ps_o, rinv)

                # ---- y = attn @ Wo.T  (token-major output) ----
                with (
                    tc.tile_pool(name="wo", bufs=3) as wo_pool,
                    tc.tile_pool(name="psy", bufs=8, space="PSUM") as psy_pool,
                    tc.tile_pool(name="yout", bufs=4) as y_pool,
                ):
                    for fc in range(8):
                        pss = [
                            psy_pool.tile([128, 512], f32, tag="ps", name="ps_y") for _ in range(4)
                        ]
                        for h in range(2):
                            wst = wo_pool.tile([128, 8, 2, 512], bf16, tag="w")
                            for lb2 in range(2):
                                nc.sync.dma_start(
                                    out=wst[:, :, lb2, :], in_=w_src("o", fc, h, lb2)
                                )
                            for i in range(16):
                                r, lb = i // 2, i % 2
                                dD = r * 4 + 2 * h + lb
                                for tb in range(4):
                                    nc.tensor.matmul(
                                        pss[tb],
                                        attnT_s[:, dD, tb * 128 : (tb + 1) * 128],
                                        wst[:, r, lb, :],
                                        start=(h == 0 and i == 0),
                                        stop=(h == 1 and i == 15),
                                    )
                        for tb in range(4):
                            yt = y_pool.tile([128, 512], bf16, tag="y")
                            nc.vector.tensor_copy(yt, pss[tb])
                            nc.sync.dma_start(
                                out=y_d[fc, tb], in_=yt
                            )


_NC_CACHE = None


def _get_program():
    global _NC_CACHE
    if _NC_CACHE is None:
        _NC_CACHE = build_program()
    return _NC_CACHE


def make_in_maps(query, key, value, mask, position_ids, Wq, Wk, Wv, Wo):
    bf16 = ml_dtypes.bfloat16
    WSH = D // NCORES

    def t_bf16(a):  # [m,n] fp32 -> [n,m] bf16, contiguous
        return np.asarray(a, np.float32).T.astype(bf16)

    wqT = t_bf16(np.asarray(Wq))
    wkT = t_bf16(np.asarray(Wk))
    wvT = t_bf16(np.asarray(Wv))
    woT = t_bf16(np.asarray(Wo))
    invf = (10000.0 ** (-np.arange(0, RD, 2, dtype=np.float32) / RD)).astype(
        np.float32
    )

    def shard(wT, b):
        # rows [b*512,(b+1)*512), packed [8 gq][512 rows][512 cols] so each
        # per-gq column slice is one contiguous AllGather input chunk
        s = wT[b * WSH : (b + 1) * WSH]  # [512, 4096]
        return np.ascontiguousarray(s.reshape(WSH, 8, 512).transpose(1, 0, 2))

    def pack_x(a):
        # [512 tok, 4096 d] fp32 -> [128 part, 32 dblk, 512 tok] bf16
        xT = t_bf16(a)  # [4096, 512]
        return np.ascontiguousarray(
            xT.reshape(32, 128, 512).transpose(1, 0, 2)
        )

    in_maps = []
    for b in range(NCORES):
        in_maps.append(
            {
                "xqT": pack_x(query[b]),
                "xkT": pack_x(key[b]),
                "xvT": pack_x(value[b]),
                # core b ships only its row-shard; the kernel AllGathers
                "wq_sh": shard(wqT, b),
                "wk_sh": shard(wkT, b),
                "wv_sh": shard(wvT, b),
                "wo_sh": shard(woT, b),
                "pos": np.ascontiguousarray(
                    np.asarray(position_ids[b], np.float32)
                ),
                "invf": invf,
                "maskin": np.ascontiguousarray(np.asarray(mask[b], np.int32)),
            }
        )
    return in_maps


def kernel(query, key, value, mask, position_ids, Wq, Wk, Wv, Wo):
    global LAST_RESULT
    nc = _get_program()
    in_maps = make_in_maps(
        query, key, value, mask, position_ids, Wq, Wk, Wv, Wo
    )
    res = run_bass_kernel_spmd(
        nc, in_maps, core_ids=list(range(NCORES)), trace=TRACE
    )
    LAST_RESULT = res
    # y comes back bf16 and fc/tb-blocked; reassemble + widen host-side.
    outs = []
    for b in range(NCORES):
        blk = np.asarray(res.results[b]["y"])  # [8 fc, 4 tb, 128, 512]
        outs.append(blk.transpose(1, 2, 0, 3).reshape(S, D))
    out = np.stack(outs, axis=0)
    return np.ascontiguousarray(out.astype(np.float32))



# revision 34
# speedup vs baseline: 1.0748x; 1.0144x over previous
"""Grouped-query attention (B=8,S=512,D=4096,G=32) on 8 trn2 cores.

Strategy: data-parallel over the batch dim — core b handles batch b —
with the WEIGHTS SHARDED across cores and AllGathered on-chip. Each
core ships only rows [c*512,(c+1)*512) of each transposed weight
(16.8MB instead of 134MB per core), cutting total host->device input
bytes ~5x; chunked, serialized AllGathers rebuild the matrices in
internal Shared DRAM while compute runs, ordered to match consumption
(Wq first, per-gq chunks, Wo last).

Per core, everything is computed in a feature-major ("transposed")
layout so no on-device transposes are needed:

  q^T[f,t] = sum_d WqT[d,f] * xqT[d,t]        (lhsT=WqT tile, rhs=xqT)
  k^T      likewise;  v[t,f] uses lhsT=xqT tile, rhs=WvT tile
  RoPE on q^T/k^T heads 0..7 (per-token angle, head g pairs with g+4)
  s^T[k,q] = kh^T_blk.T @ qh^T   (per head, 4 k-blocks of 128)
  w^T      = exp(s^T + maskbias) (no max-subtraction; logits are O(10))
  o^T[dh,q]= sum_kb vh_blk.T @ w^T_blk        (lhsT=vh block)
  denom    = DVE-sum of w^T blocks -> GpSimd partition_all_reduce
  r        = 1/denom on one row -> GpSimd partition_broadcast
  attn^T   = o^T * r  (bf16)
  y[t,f]   = sum_D attnT_blk.T @ WoT tile

The v-projection and attention are interleaved per 4-head block:
attention needs no gathered weights, so it is filler work the
scheduler can run whenever a weight AllGather chunk lags the PE.
Matmuls run in bf16 (fp32 PSUM accumulation); softmax math in fp32;
y returns bf16 (host widens) to halve the d2h bytes.
Host side only shards, transposes (layout), casts dtypes and gathers.
"""

import math

import numpy as np
import ml_dtypes

import concourse.bass as bass
import concourse.mybir as mybir
import concourse.tile as tile
from concourse import bacc
from concourse.bass_utils import run_bass_kernel_spmd
from concourse.tile_rust import add_dep_helper
from concourse import bass_isa

B, S, D = 8, 512, 4096
G, DH = 32, 128
RD = 1024
ALPHA = 1.0 / math.sqrt(DH)
PI = math.pi
NCORES = 8
DT = mybir.dt
AF = mybir.ActivationFunctionType
ALU = mybir.AluOpType

# set by test.py to capture a profile
TRACE = False
LAST_RESULT = None


def _range_reduce(nc, ang, mtmp):
    """In-place reduce ang (>=0, < ~7*pi/2) into (-pi, pi] mod 2*pi."""
    for _ in range(3):
        # mtmp = (ang > pi) * 2pi ; ang -= mtmp
        nc.vector.tensor_scalar(mtmp, ang, PI, 2.0 * PI, ALU.is_gt, ALU.mult)
        nc.vector.tensor_sub(ang, ang, mtmp)


def build_program():
    # Bacc (not plain Bass): its compile pipeline lowers multi-sem waits to
    # the single ISA wait slot; plain Bass BIR fails walrus codegen.
    nc = bacc.Bacc(
        "TRN2", target_bir_lowering=False, debug=False, num_devices=NCORES
    )
    bf16 = DT.bfloat16
    f32 = DT.float32

    # Weights arrive SHARDED: core c holds rows [c*512, (c+1)*512) of each
    # transposed weight (1/8 of the bytes), packed host-side as
    # [8 gq][512 rows][512 cols] so a per-gq column slice is contiguous.
    # On-chip chunked AllGathers rebuild the full matrices in internal
    # DRAM — host->device traffic for weights drops 8x vs replication,
    # and chunking lets the first projection start after the first small
    # gather instead of a full-matrix one.
    WSH = D // NCORES  # 512 rows per shard
    # x inputs pre-packed host-side to [128 part, 32 dblk, S]: one fully
    # contiguous 4MB DMA each instead of strided gathers.
    xqT_d = nc.declare_dram_parameter("xqT", [128, 32, S], bf16, isOutput=False)
    xkT_d = nc.declare_dram_parameter("xkT", [128, 32, S], bf16, isOutput=False)
    xvT_d = nc.declare_dram_parameter("xvT", [128, 32, S], bf16, isOutput=False)
    wq_sh = nc.declare_dram_parameter("wq_sh", [8, WSH, 512], bf16, isOutput=False)
    wk_sh = nc.declare_dram_parameter("wk_sh", [8, WSH, 512], bf16, isOutput=False)
    wv_sh = nc.declare_dram_parameter("wv_sh", [8, WSH, 512], bf16, isOutput=False)
    wo_sh = nc.declare_dram_parameter("wo_sh", [8, WSH, 512], bf16, isOutput=False)
    pos_d = nc.declare_dram_parameter("pos", [S], f32, isOutput=False)
    invf_d = nc.declare_dram_parameter("invf", [S], f32, isOutput=False)
    mask_d = nc.declare_dram_parameter("maskin", [S], DT.int32, isOutput=False)
    # y blocked [fc, tb, 128, 512] so each store is one contiguous DMA;
    # host reassembles to [S, D].
    y_d = nc.declare_dram_parameter("y", [8, 4, 128, 512], bf16, isOutput=True)

    RG = [list(range(NCORES))]

    with tile.TileContext(nc) as tc:
        with tc.tile_pool(name="dram", bufs=1, space="DRAM") as dram:
            # chunk sizes (in gq blocks of 512 cols) per weight: finest for
            # Wq (gates the first matmuls), coarser later to save CC floors.
            # AGs are chained (serialized) so the first chunk's data phase
            # is not delayed behind later collectives' handshakes.
            gathered = {}  # (which, gq) -> (tile, local_j)
            prev_cc = [None]

            def gather_weight(which, sh_d, chunks):
                gq0 = 0
                for ci, csz in enumerate(chunks):
                    bounce = dram.tile(
                        [csz, WSH, 512], bf16,
                        tag=f"b{which}{ci}", name=f"bounce_{which}{ci}",
                    )
                    g_t = dram.tile(
                        [NCORES, csz, WSH, 512], bf16, addr_space="Shared",
                        tag=f"g{which}{ci}", name=f"gath_{which}{ci}",
                    )
                    nc.gpsimd.dma_start(
                        out=bounce, in_=sh_d[gq0 : gq0 + csz]
                    )
                    cc = nc.gpsimd.collective_compute(
                        "AllGather",
                        ALU.bypass,
                        replica_groups=RG,
                        ins=[bounce.opt()],
                        outs=[g_t.opt()],
                    )
                    if prev_cc[0] is not None:
                        add_dep_helper(cc.ins, prev_cc[0].ins, reason="serialize AGs")
                    prev_cc[0] = cc
                    for j in range(csz):
                        gathered[(which, gq0 + j)] = (g_t, j)
                    gq0 += csz

            gather_weight("q", wq_sh, (1, 1, 2, 2, 2))
            gather_weight("k", wk_sh, (2, 2, 2, 2))
            gather_weight("v", wv_sh, (8,))
            gather_weight("o", wo_sh, (8,))

            def w_tile(which, gq, db):
                """[128, 512] tile of wT[db*128:(db+1)*128, gq cols]."""
                g_t, j = gathered[(which, gq)]
                r, lb = db // 4, db % 4
                return g_t[r, j, lb * 128 : (lb + 1) * 128, :]

            def w_src(which, gq, h, lb):
                """Strided 3-dim AP [128, 8, 512]: d-blocks {r*4 + 2h + lb}
                for r in 0..7 of column block gq (DMA APs max 3 dims)."""
                g_t, j = gathered[(which, gq)]
                r0 = h * 256 + lb * 128
                src = g_t[:, j, r0 : r0 + 128, :]
                return src.rearrange("r p f -> p r f", p=128)

            _build_body(nc, tc, xqT_d, xkT_d, xvT_d, w_src, w_tile,
                        pos_d, invf_d, mask_d, y_d)
    # Bacc lowering: splits multi-sem waits into the single ISA wait slot,
    # allocates registers, fuses nops. Required before walrus codegen.
    nc.compile()
    return nc


def _build_body(nc, tc, xqT_d, xkT_d, xvT_d, w_src, w_tile,
                pos_d, invf_d, mask_d, y_d):
    bf16 = DT.bfloat16
    f32 = DT.float32
    if True:
        with tc.tile_pool(name="persist", bufs=1) as persist:
            # trig tiles, broadcast over partitions: [128, S]
            sin_t = persist.tile([128, S], f32, tag="sin")
            cos_t = persist.tile([128, S], f32, tag="cos")
            sinq_t = persist.tile([128, S], f32, tag="sinq")
            cosq_t = persist.tile([128, S], f32, tag="cosq")
            maskb = persist.tile([128, 4], f32, tag="maskb")
            ones_bf = persist.tile([128, 1], bf16, tag="ones_bf")
            ones_f1 = persist.tile([1, 128], f32, tag="ones_f1")
            qT_s = persist.tile([128, G, S], bf16, tag="qT")
            kT_s = persist.tile([128, G, S], bf16, tag="kT")
            v_s = persist.tile([128, 4, G, DH], bf16, tag="v")

            nc.vector.memset(ones_bf, 1.0)
            nc.vector.memset(ones_f1, 1.0)

            # ---- setup: trig + mask ----
            # Trig is computed on one partition, then broadcast to all 128
            # partitions with a rank-1 matmul (ones [1,128] x row [1,S]).
            with (
                tc.tile_pool(name="setup", bufs=1) as setup,
                tc.tile_pool(name="ps_setup", bufs=2, space="PSUM") as ps_setup,
            ):
                pos1 = setup.tile([1, S], f32, tag="pos1")
                invf1 = setup.tile([1, S], f32, tag="invf1")
                angc = setup.tile([1, S], f32, tag="angc")
                mtmp = setup.tile([1, S], f32, tag="mtmp")
                mi = setup.tile([128, 4], DT.int32, tag="mi")
                mf = setup.tile([128, 4], f32, tag="mf")

                nc.sync.dma_start(out=pos1, in_=pos_d[None, :])
                nc.sync.dma_start(out=invf1, in_=invf_d[None, :])
                # angles for sin, reuse pos1 as buffer for sin-angles
                angs = pos1
                nc.vector.tensor_mul(angs, pos1, invf1)
                nc.vector.tensor_scalar_add(angc, angs, PI / 2.0)
                _range_reduce(nc, angs, mtmp)
                _range_reduce(nc, angc, mtmp)
                nc.scalar.activation(angs, angs, AF.Sin)
                nc.scalar.activation(angc, angc, AF.Sin)
                ps_sin = ps_setup.tile([128, S], f32, tag="b", name="ps_sin")
                ps_cos = ps_setup.tile([128, S], f32, tag="b", name="ps_cos")
                nc.tensor.matmul(ps_sin, ones_f1, angs, start=True, stop=True)
                nc.tensor.matmul(ps_cos, ones_f1, angc, start=True, stop=True)
                nc.scalar.copy(sin_t, ps_sin)
                nc.scalar.copy(cos_t, ps_cos)
                nc.scalar.mul(sinq_t, ps_sin, ALPHA)
                nc.scalar.mul(cosq_t, ps_cos, ALPHA)

                nc.sync.dma_start(out=mi, in_=mask_d[:].rearrange("(b p) -> p b", p=128))
                nc.vector.tensor_copy(mf, mi)
                # maskb = m * 1e9 - 1e9  (0 where m==1, -1e9 where m==0)
                nc.vector.tensor_scalar(
                    maskb, mf, 1.0e9, 1.0e9, ALU.mult, ALU.subtract
                )

            # ---- q and k projections (output feature-major) + RoPE ----
            for which, xT_d, outT, ct, st in (
                ("q", xqT_d, qT_s, cosq_t, sinq_t),
                ("k", xkT_d, kT_s, cos_t, sin_t),
            ):
                scale = ALPHA if which == "q" else 1.0
                with (
                    tc.tile_pool(name=f"x{which}", bufs=1) as xin_pool,
                    tc.tile_pool(name=f"w{which}", bufs=3) as w_pool,
                    tc.tile_pool(name=f"ps{which}", bufs=8, space="PSUM") as ps_pool,
                    tc.tile_pool(name=f"rope{which}", bufs=1) as rope_pool,
                    tc.tile_pool(name=f"rt{which}", bufs=4) as rtmp_pool,
                ):
                    xT_s = xin_pool.tile([128, 32, S], bf16, tag="xT")
                    nc.sync.dma_start(out=xT_s, in_=xT_d[:])
                    rope_f32 = rope_pool.tile([128, 8, S], bf16, tag="rope")
                    for gq in range(8):
                        pss = [
                            ps_pool.tile([128, S], f32, tag="ps", name="ps_qk") for _ in range(4)
                        ]
                        for h in range(2):
                            wst = w_pool.tile([128, 8, 2, 512], bf16, tag="w")
                            for lb2 in range(2):
                                nc.sync.dma_start(
                                    out=wst[:, :, lb2, :],
                                    in_=w_src(which, gq, h, lb2),
                                )
                            for i in range(16):
                                r, lb = i // 2, i % 2
                                db = r * 4 + 2 * h + lb
                                for g4 in range(4):
                                    nc.tensor.matmul(
                                        pss[g4],
                                        wst[:, r, lb, g4 * 128 : (g4 + 1) * 128],
                                        xT_s[:, db, :],
                                        start=(h == 0 and i == 0),
                                        stop=(h == 1 and i == 15),
                                    )
                        for g4 in range(4):
                            g = gq * 4 + g4
                            if g < 8:
                                # RoPE heads: stash fp32
                                nc.vector.tensor_copy(rope_f32[:, g, :], pss[g4])
                            elif which == "q":
                                nc.scalar.activation(
                                    outT[:, g, :], pss[g4], AF.Copy, scale=scale
                                )
                            else:
                                nc.vector.tensor_copy(outT[:, g, :], pss[g4])
                    # RoPE: head g pairs with head g+4 (per-token scalar angle)
                    for g in range(4):
                        a = rope_f32[:, g, :]
                        b = rope_f32[:, g + 4, :]
                        t1 = rtmp_pool.tile([128, S], f32, tag="t")
                        t2 = rtmp_pool.tile([128, S], f32, tag="t")
                        t3 = rtmp_pool.tile([128, S], f32, tag="t")
                        t4 = rtmp_pool.tile([128, S], f32, tag="t")
                        nc.vector.tensor_mul(t1, a, ct)
                        nc.vector.tensor_mul(t2, b, st)
                        nc.vector.tensor_sub(outT[:, g, :], t1, t2)
                        nc.vector.tensor_mul(t3, b, ct)
                        nc.vector.tensor_mul(t4, a, st)
                        nc.vector.tensor_add(outT[:, g + 4, :], t3, t4)

            # ---- v projection + attention, interleaved per fc block ----
            # Attention needs no gathered weights, so it is PE filler work
            # the scheduler can run whenever v's AllGathered chunks lag.
            # Softmax denominators go through GpSimd partition_all_reduce
            # (idle engine) instead of ones-matmuls: frees 4 PSUM banks
            # and 5 small matmuls per head.
            with tc.tile_pool(name="attn", bufs=1) as attn_pool:
                attnT_s = attn_pool.tile([128, G, S], bf16, tag="attnT")
                with (
                    tc.tile_pool(name="xv", bufs=1) as xin_pool,
                    tc.tile_pool(name="wv", bufs=12) as w_pool,
                    tc.tile_pool(name="psv", bufs=4, space="PSUM") as ps_pool,
                    tc.tile_pool(name="wexp", bufs=6) as wexp_pool,
                    tc.tile_pool(name="rsm", bufs=6) as rpool,
                    tc.tile_pool(name="ps_s", bufs=2, space="PSUM") as ps_s_pool,
                    tc.tile_pool(name="ps_o", bufs=2, space="PSUM") as ps_o_pool,
                ):
                    xT_s = xin_pool.tile([128, 32, S], bf16, tag="xT")
                    nc.sync.dma_start(out=xT_s, in_=xvT_d[:])
                    for fc in range(8):
                        pss = [ps_pool.tile([128, 512], f32, tag="ps", name="ps_v") for _ in range(4)]
                        for d in range(32):
                            wt = w_pool.tile([128, 512], bf16, tag="w")
                            nc.sync.dma_start(out=wt, in_=w_tile("v", fc, d))
                            for tb in range(4):
                                nc.tensor.matmul(
                                    pss[tb],
                                    xT_s[:, d, tb * 128 : (tb + 1) * 128],
                                    wt,
                                    start=(d == 0),
                                    stop=(d == 31),
                                )
                        for tb in range(4):
                            nc.vector.tensor_copy(
                                v_s[:, tb, fc * 4 : (fc + 1) * 4, :], pss[tb]
                            )
                        for g in range(fc * 4, fc * 4 + 4):
                            ps_o = ps_o_pool.tile([128, S], f32, tag="o")
                            wbs = []
                            for kb in range(4):
                                ps_sc = ps_s_pool.tile([128, S], f32, tag="s")
                                nc.tensor.matmul(
                                    ps_sc,
                                    kT_s[:, g, kb * 128 : (kb + 1) * 128],
                                    qT_s[:, g, :],
                                    start=True,
                                    stop=True,
                                )
                                wb = wexp_pool.tile([128, S], bf16, tag="w")
                                nc.scalar.activation(
                                    wb, ps_sc, AF.Exp,
                                    bias=maskb[:, kb : kb + 1], scale=1.0,
                                )
                                nc.tensor.matmul(
                                    ps_o,
                                    v_s[:, kb, g, :],
                                    wb,
                                    start=(kb == 0),
                                    stop=(kb == 3),
                                )
                                wbs.append(wb)
                            s01 = rpool.tile([128, S], f32, tag="t", bufs=4)
                            s23 = rpool.tile([128, S], f32, tag="t", bufs=4)
                            stot = rpool.tile([128, S], f32, tag="t", bufs=4)
                            nc.vector.tensor_add(s01, wbs[0], wbs[1])
                            nc.vector.tensor_add(s23, wbs[2], wbs[3])
                            nc.vector.tensor_add(stot, s01, s23)
                            rsum = rpool.tile([128, S], f32, tag="t", bufs=4)
                            nc.gpsimd.partition_all_reduce(
                                rsum, stot, 128, bass_isa.ReduceOp.add
                            )
                            # reciprocal on ONE partition row (iterative op,
                            # 8x per-element cost), then GpSimd-broadcast
                            rrow = rpool.tile([1, S], f32, tag="rr", bufs=2)
                            nc.vector.reciprocal(rrow, rsum[0:1, :])
                            rinv = rpool.tile([128, S], f32, tag="ri", bufs=2)
                            nc.gpsimd.partition_broadcast(rinv, rrow)
                            nc.vector.tensor_mul(attnT_s[:, g, :], ps_o, rinv)

                # ---- y = attn @ Wo.T  (token-major output) ----
                with (
                    tc.tile_pool(name="wo", bufs=3) as wo_pool,
                    tc.tile_pool(name="psy", bufs=8, space="PSUM") as psy_pool,
                    tc.tile_pool(name="yout", bufs=4) as y_pool,
                ):
                    for fc in range(8):
                        pss = [
                            psy_pool.tile([128, 512], f32, tag="ps", name="ps_y") for _ in range(4)
                        ]
                        for h in range(2):
                            wst = wo_pool.tile([128, 8, 2, 512], bf16, tag="w")
                            for lb2 in range(2):
                                nc.sync.dma_start(
                                    out=wst[:, :, lb2, :], in_=w_src("o", fc, h, lb2)
                                )
                            for i in range(16):
                                r, lb = i // 2, i % 2
                                dD = r * 4 + 2 * h + lb
                                for tb in range(4):
                                    nc.tensor.matmul(
                                        pss[tb],
                                        attnT_s[:, dD, tb * 128 : (tb + 1) * 128],
                                        wst[:, r, lb, :],
                                        start=(h == 0 and i == 0),
                                        stop=(h == 1 and i == 15),
                                    )
                        for tb in range(4):
                            yt = y_pool.tile([128, 512], bf16, tag="y")
                            nc.vector.tensor_copy(yt, pss[tb])
                            nc.sync.dma_start(
                                out=y_d[fc, tb], in_=yt
                            )


_NC_CACHE = None


def _get_program():
    global _NC_CACHE
    if _NC_CACHE is None:
        _NC_CACHE = build_program()
    return _NC_CACHE


def make_in_maps(query, key, value, mask, position_ids, Wq, Wk, Wv, Wo):
    bf16 = ml_dtypes.bfloat16
    WSH = D // NCORES

    def t_bf16(a):  # [m,n] fp32 -> [n,m] bf16, contiguous
        return np.asarray(a, np.float32).T.astype(bf16)

    wqT = t_bf16(np.asarray(Wq))
    wkT = t_bf16(np.asarray(Wk))
    wvT = t_bf16(np.asarray(Wv))
    woT = t_bf16(np.asarray(Wo))
    invf = (10000.0 ** (-np.arange(0, RD, 2, dtype=np.float32) / RD)).astype(
        np.float32
    )

    def shard(wT, b):
        # rows [b*512,(b+1)*512), packed [8 gq][512 rows][512 cols] so each
        # per-gq column slice is one contiguous AllGather input chunk
        s = wT[b * WSH : (b + 1) * WSH]  # [512, 4096]
        return np.ascontiguousarray(s.reshape(WSH, 8, 512).transpose(1, 0, 2))

    def pack_x(a):
        # [512 tok, 4096 d] fp32 -> [128 part, 32 dblk, 512 tok] bf16
        xT = t_bf16(a)  # [4096, 512]
        return np.ascontiguousarray(
            xT.reshape(32, 128, 512).transpose(1, 0, 2)
        )

    in_maps = []
    for b in range(NCORES):
        in_maps.append(
            {
                "xqT": pack_x(query[b]),
                "xkT": pack_x(key[b]),
                "xvT": pack_x(value[b]),
                # core b ships only its row-shard; the kernel AllGathers
                "wq_sh": shard(wqT, b),
                "wk_sh": shard(wkT, b),
                "wv_sh": shard(wvT, b),
                "wo_sh": shard(woT, b),
                "pos": np.ascontiguousarray(
                    np.asarray(position_ids[b], np.float32)
                ),
                "invf": invf,
                "maskin": np.ascontiguousarray(np.asarray(mask[b], np.int32)),
            }
        )
    return in_maps


def kernel(query, key, value, mask, position_ids, Wq, Wk, Wv, Wo):
    global LAST_RESULT
    nc = _get_program()
    in_maps = make_in_maps(
        query, key, value, mask, position_ids, Wq, Wk, Wv, Wo
    )
    res = run_bass_kernel_spmd(
        nc, in_maps, core_ids=list(range(NCORES)), trace=TRACE
    )
    LAST_RESULT = res
    # y comes back bf16 and fc/tb-blocked; reassemble + widen host-side.
    outs = []
    for b in range(NCORES):
        blk = np.asarray(res.results[b]["y"])  # [8 fc, 4 tb, 128, 512]
        outs.append(blk.transpose(1, 2, 0, 3).reshape(S, D))
    out = np.stack(outs, axis=0)
    return np.ascontiguousarray(out.astype(np.float32))

